# revision 2
# baseline (speedup 1.0000x reference)
"""ConvNAT (conv QKV + 2D dilated neighborhood attention) on 8 trn2 cores.

Sharding: core = (batch b, H-slab of 24 rows).  Each core computes conv
q/k/v for its slab (+12-row halo recompute) and the attention for its 24
output rows.  H-border rows (h<6, h>=90), whose NATTEN windows are clamped
and would break SPMD program uniformity, are computed on the host in numpy
and overwrite the device output.

Attention on device (per output row h):
  logits(96q x 7kr x 96kc) = Q_ext^T K_ext   (f32r matmuls, N>=256)
    Q_ext/K_ext = [conv channels (64) | 7 "h-distance" channels that
    reproduce scale*pe_h.pe_h(h-h') exactly for |h-h'|<=6 via a cosine
    interpolation], q side pre-scaled by 128^-0.5.
  DVE adds WBIAS[w,kc] = scale*pe_w.pe_w + (0 / -30000) W-band+parity mask.
  ACT exp -> P (bf16) with accum_out giving the softmax denominator free.
  PE transposes P per key row; AV = sum_i P_i^T V_i^T (bf16) into psum;
  DVE multiplies by 1/den during psum evacuation.
"""
import os
import re
import sys

sys.path.insert(0, '/opt/trn_rl_repo')

import numpy as np

import concourse.bass as bass
from concourse import mybir
from concourse.tile import TileContext
from concourse.masks import make_identity
from bass_rust import ScopedClock, VectorClock

F32 = mybir.dt.float32
F32R = mybir.dt.float32r
BF16 = mybir.dt.bfloat16

B, CIN, H, W = 2, 64, 96, 96
CI, CO = 64, 128
KS, DIL = 7, 2
SCALE = float(CI * 2) ** -0.5  # Cqk = 128 after pe concat
HS = 24          # rows per core
NH = 4           # h-slabs
NCORES = 8
NDIST = 7        # h-distance channels
CH = 64 + NDIST  # contraction channels
KV = 48          # k/v rows per core (24 + 12 halo each side, unclamped)
XR = 50          # x slab rows (KV + conv halo)
NEG = -30000.0

# ---------------------------------------------------------------- compat ---
MAX_WAITS = 1


def _patched_drain(self, tick_clock, wait_clock):
    nc = self.nc
    ticks = [int(v) for v in re.findall(r'\d+', repr(tick_clock.global_clock))]
    for i in range(0, len(ticks), MAX_WAITS):
        chunk = [0] * len(ticks)
        chunk[i:i + MAX_WAITS] = ticks[i:i + MAX_WAITS]
        if any(chunk):
            probe = nc.sync.nop()
            wait_clock.add_sem_waits(probe.ins, ScopedClock({None: VectorClock(chunk)}))
    nc.sync.drain()
    nc.all_engine_barrier()
    popped = nc._tile_sem_poison_stack.pop()
    assert popped is self._sem_poison
    nc.clear_and_free_semaphores(list(self.sems.allocated().values()))
    nc.all_engine_barrier()


TileContext._drain_and_barrier = _patched_drain


def _split_excess_waits(nc, max_waits=MAX_WAITS):
    n_split = 0
    for fn in nc.m.functions:
        for bb in fn.blocks:
            out = []
            changed = False
            for inst in bb.instructions:
                si = inst.sync_info
                waits = list(si.on_wait) if si and si.on_wait else []
                if len(waits) > max_waits:
                    extra = waits[:-max_waits]
                    for j in range(0, len(extra), max_waits):
                        nop = mybir.InstNoOp(name=f"{inst.name}-ws{j}", ins=[], outs=[])
                        nop.engine = inst.engine
                        nop.sync_info = mybir.SyncInfo(
                            on_wait=extra[j:j + max_waits], on_update=[])
                        out.append(nop)
                    si.on_wait = waits[-max_waits:]
                    changed = True
                    n_split += 1
                out.append(inst)
            if changed:
                bb.instructions = out
    return n_split


# ------------------------------------------------------------- host math ---
def _sincos(length, dim):
    half = dim // 2
    inv_freq = 1.0 / (10000.0 ** (np.arange(half, dtype=np.float64) * 2.0 / dim))
    ang = np.arange(length, dtype=np.float64)[:, None] * inv_freq[None, :]
    return np.concatenate([np.sin(ang), np.cos(ang)], axis=-1)  # (L, dim)


def _na_indices(L, K, D):
    i = np.arange(L)
    g = i % D
    r = i // D
    Lg = (L - g + D - 1) // D
    start = np.clip(r - K // 2, 0, Lg - K)
    return g[:, None] + (start[:, None] + np.arange(K)[None, :]) * D  # (L, K)


def _hdist_channels():
    """QD,KD (NDIST, 96): sum_m QD[m,h]*KD[m,h'] == SCALE*pe_h[h].pe_h[h']
    exactly for even |h-h'| <= 6."""
    pe = _sincos(H, 32)           # (96, 32)
    inv_freq = 1.0 / (10000.0 ** (np.arange(16, dtype=np.float64) * 2.0 / 32))
    dv = np.array([0., 2., 4., 6.])
    g = SCALE * np.cos(dv[:, None] * inv_freq[None, :]).sum(1)  # exact pe.pe(d)
    th = np.arange(4, dtype=np.float64) * (np.pi / 6.0)
    M = np.cos(dv[:, None] * th[None, :])                        # (4, 4)
    b = np.linalg.solve(M, g)
    hh = np.arange(H, dtype=np.float64)
    QD = np.zeros((NDIST, H))
    KD = np.zeros((NDIST, H))
    QD[0] = b[0]
    KD[0] = 1.0
    for m in range(1, 4):
        QD[2 * m - 1] = b[m] * np.cos(th[m] * hh)
        QD[2 * m] = b[m] * np.sin(th[m] * hh)
        KD[2 * m - 1] = np.cos(th[m] * hh)
        KD[2 * m] = np.sin(th[m] * hh)
    # verify
    got = QD.T @ KD
    ref = SCALE * (pe @ pe.T)
    for d in (-6, -4, -2, 0, 2, 4, 6):
        idx = np.arange(max(0, -d), min(H, H - d))
        err = np.abs(got[idx, idx + d] - ref[idx, idx + d]).max()
        assert err < 1e-6, (d, err)
    return QD.astype(np.float32), KD.astype(np.float32)


def _wbias():
    pe = _sincos(W, 32)
    idx_w = _na_indices(W, KS, DIL)   # (96, 7)
    wb = np.full((W, W), NEG, dtype=np.float64)
    dot = SCALE * (pe @ pe.T)
    for w in range(W):
        wb[w, idx_w[w]] = dot[w, idx_w[w]]
    return wb.astype(np.float32)


def _conv_np(x, w, bias, rows):
    """NCHW 3x3 pad-1 conv evaluated at `rows` -> (B, len(rows), 96, Cout)."""
    Bn, Cin, Hn, Wn = x.shape
    xp = np.zeros((Bn, Cin, Hn + 2, Wn + 2), dtype=np.float64)
    xp[:, :, 1:-1, 1:-1] = x
    rows = np.asarray(rows)
    acc = np.zeros((Bn, len(rows), Wn, w.shape[0]), dtype=np.float64)
    for ky in range(3):
        for kx in range(3):
            xs = xp[:, :, rows + ky, :][:, :, :, kx:kx + Wn]  # (B,C,R,W)
            acc += np.einsum('bcrw,oc->brwo', xs, w[:, :, ky, kx].astype(np.float64))
    return acc + bias[None, None, None, :].astype(np.float64)


def _host_border(x, wq, bq, wk, bk, wv, bv):
    """Reference computation for the clamped border rows. -> dict h -> (B,96,128)."""
    border_h = list(range(0, 6)) + list(range(90, 96))
    kv_rows = sorted(set(np.concatenate([_na_indices(H, KS, DIL)[h] for h in border_h])))
    kv_rows = np.asarray(kv_rows)
    q_c = _conv_np(x, wq, bq, np.asarray(border_h))     # (B, 12, 96, 64)
    k_c = _conv_np(x, wk, bk, kv_rows)                  # (B, R, 96, 64)
    v_c = _conv_np(x, wv, bv, kv_rows)                  # (B, R, 96, 128)
    kv_pos = {r: i for i, r in enumerate(kv_rows)}
    pe_h = _sincos(H, 32)
    pe_w = _sincos(W, 32)
    idx_h = _na_indices(H, KS, DIL)
    idx_w = _na_indices(W, KS, DIL)
    out = {}
    for bi, h in enumerate(border_h):
        pe_q = np.concatenate([np.repeat(pe_h[h][None], W, 0), pe_w], axis=1)  # (96,64)
        q = np.concatenate([q_c[:, bi], np.repeat(pe_q[None], B, 0)], axis=2)  # (B,96,128)
        rows = [kv_pos[r] for r in idx_h[h]]
        kk = k_c[:, rows]                                   # (B,7,96,64)
        vv = v_c[:, rows]                                   # (B,7,96,128)
        pe_k = np.concatenate(
            [np.repeat(pe_h[idx_h[h]][:, None, :], W, 1),
             np.repeat(pe_w[None], KS, 0)], axis=2)         # (7,96,64)
        kk = np.concatenate([kk, np.repeat(pe_k[None], B, 0)], axis=3)  # (B,7,96,128)
        kn = kk[:, :, idx_w]                                # (B,7,96,7,128)
        vn = vv[:, :, idx_w]
        logits = SCALE * np.einsum('bwc,biwjc->bwij', q, kn)   # (B,96,7,7)
        m = logits.reshape(B, W, -1).max(-1)
        p = np.exp(logits - m[:, :, None, None])
        p /= p.reshape(B, W, -1).sum(-1)[:, :, None, None]
        out[h] = np.einsum('bwij,biwjc->bwc', p, vn)        # (B,96,128)
    return out


# ------------------------------------------------------------ bass build ---
_CACHE = {}


def _build_program():
    if 'nc' in _CACHE:
        return _CACHE['nc']
    nc = bass.Bass('TRN2')
    xs = nc.dram_tensor('xs', (64, XR, 98), F32R, kind='ExternalInput')
    wqk2 = nc.dram_tensor('wqk2', (3, 128, 128), F32R, kind='ExternalInput')
    wqk1 = nc.dram_tensor('wqk1', (3, 64, 128), F32R, kind='ExternalInput')
    wv2 = nc.dram_tensor('wv2', (3, 128, 128), F32R, kind='ExternalInput')
    wv1 = nc.dram_tensor('wv1', (3, 64, 128), F32R, kind='ExternalInput')
    qkb = nc.dram_tensor('qkb', (128, 1), F32, kind='ExternalInput')
    vb = nc.dram_tensor('vb', (128, 1), F32, kind='ExternalInput')
    qd = nc.dram_tensor('qd', (NDIST, HS * 96), F32R, kind='ExternalInput')
    kd = nc.dram_tensor('kd', (NDIST, KV * 96), F32R, kind='ExternalInput')
    wbt = nc.dram_tensor('wbt', (96, 96), F32, kind='ExternalInput')
    o = nc.dram_tensor('o', (HS, 96, 128), F32, kind='ExternalOutput')

    with TileContext(nc) as tc:
        with tc.tile_pool(name='persist', bufs=1) as pp:
            x2 = pp.tile([128, XR, 98], F32R)
            nc.sync.dma_start(out=x2[0:64], in_=xs[:])
            nc.sync.dma_start(out=x2[64:128, 0:XR - 1, :], in_=xs[:, 1:XR, :])
            w_qk2 = pp.tile([128, 3, 128], F32R)
            nc.sync.dma_start(out=w_qk2, in_=wqk2[:].rearrange('t p n -> p t n'))
            w_qk1 = pp.tile([64, 3, 128], F32R)
            nc.sync.dma_start(out=w_qk1, in_=wqk1[:].rearrange('t p n -> p t n'))
            w_v2 = pp.tile([128, 3, 128], F32R)
            nc.sync.dma_start(out=w_v2, in_=wv2[:].rearrange('t p n -> p t n'))
            w_v1 = pp.tile([64, 3, 128], F32R)
            nc.sync.dma_start(out=w_v1, in_=wv1[:].rearrange('t p n -> p t n'))
            qkbias = pp.tile([128, 1], F32)
            nc.sync.dma_start(out=qkbias, in_=qkb[:])
            vbias = pp.tile([128, 1], F32)
            nc.sync.dma_start(out=vbias, in_=vb[:])
            wb = pp.tile([96, 96], F32)
            nc.sync.dma_start(out=wb, in_=wbt[:])
            ident = pp.tile([128, 128], BF16)
            make_identity(nc, ident)

            q_ext = pp.tile([CH, HS * 96], F32R)
            k_ext = pp.tile([CH, KV * 96], F32R)
            vsb = pp.tile([128, KV * 96], BF16)
            vt = pp.tile([96, KV * 128], BF16)
            nc.sync.dma_start(out=q_ext[64:CH, :], in_=qd[:])
            nc.sync.dma_start(out=k_ext[64:CH, :], in_=kd[:])

            # ------------------------------------------------ convolution --
            # kv slab rows 0..47 = image rows h0-12 .. h0+35 (zero-padded x).
            # x slab row 0 = image h0-13; conv for kv row r uses slab rows
            # r, r+1, r+2 (ky=0,1,2).  Pair taps (ky=0,1) via the doubled-x
            # tile; ky=2 is a K=64 matmul on the low partitions.
            qoff = 12  # q rows are kv rows 12..35  (h0 .. h0+23)
            for half in range(2):
                for which in range(2):  # 0 = qk, 1 = v
                    with tc.tile_pool(name='cps', bufs=6, space='PSUM') as cps:
                        w2 = w_qk2 if which == 0 else w_v2
                        w1 = w_qk1 if which == 0 else w_v1
                        psl = []
                        for rg in range(6):
                            t_c = cps.tile([128, 4, 96], F32, tag='c')
                            psl.append(t_c)
                        for kx in range(3):
                            for rg in range(6):
                                r0 = half * 24 + rg * 4
                                nc.tensor.matmul(
                                    psl[rg][:], w2[:, kx, :],
                                    x2[:, r0:r0 + 4, kx:kx + 96],
                                    start=(kx == 0), stop=False)
                            for rg in range(6):
                                r0 = half * 24 + rg * 4
                                nc.tensor.matmul(
                                    psl[rg][:], w1[:, kx, :],
                                    x2[0:64, r0 + 2:r0 + 6, kx:kx + 96],
                                    start=False, stop=(kx == 2))
                        for rg in range(6):
                            r0 = half * 24 + rg * 4
                            if which == 0:
                                if qoff <= r0 < qoff + HS:
                                    nc.scalar.activation(
                                        out=q_ext[0:64, (r0 - qoff) * 96:(r0 - qoff + 4) * 96],
                                        in_=psl[rg][0:64], func=mybir.ActivationFunctionType.Identity,
                                        bias=qkbias[0:64])
                                nc.vector.tensor_scalar(
                                    out=k_ext[0:64, r0 * 96:(r0 + 4) * 96],
                                    in0=psl[rg][64:128], scalar1=qkbias[64:128],
                                    scalar2=None, op0=mybir.AluOpType.add)
                            else:
                                if rg % 2 == 0:
                                    nc.scalar.activation(
                                        out=vsb[:, r0 * 96:(r0 + 4) * 96],
                                        in_=psl[rg][:], func=mybir.ActivationFunctionType.Identity,
                                        bias=vbias[:])
                                else:
                                    nc.vector.tensor_scalar(
                                        out=vsb[:, r0 * 96:(r0 + 4) * 96],
                                        in0=psl[rg][:], scalar1=vbias[:],
                                        scalar2=None, op0=mybir.AluOpType.add)

            # ------------------------------------------------- V^T --------
            with tc.tile_pool(name='tps', bufs=2, space='PSUM') as tps:
                for grp in range(12):
                    pst = tps.tile([96, 4, 128], BF16, tag='t')
                    for rr in range(4):
                        r = grp * 4 + rr
                        nc.tensor.transpose(
                            pst[:, rr, :], vsb[:, r * 96:(r + 1) * 96], ident)
                    if grp % 2 == 0:
                        nc.scalar.copy(vt[:, grp * 512:(grp + 1) * 512], pst[:])
                    else:
                        nc.vector.tensor_copy(
                            out=vt[:, grp * 512:(grp + 1) * 512], in_=pst[:])

            # ---------------------------------------------- attention -----
            wb_ap = wb[:, :]
            wb_b = bass.AP(tensor=wb_ap.tensor, offset=wb_ap.offset,
                           ap=[wb_ap.ap[0], [0, KS], wb_ap.ap[1]])
            with tc.tile_pool(name='aps', bufs=2, space='PSUM') as aps, \
                 tc.tile_pool(name='tp2', bufs=1, space='PSUM') as tp2, \
                 tc.tile_pool(name='ops', bufs=2, space='PSUM') as ops, \
                 tc.tile_pool(name='att', bufs=2) as att:
                for j in range(HS):
                    kr0 = j + 6  # kv slab row of first key row (interior pattern)
                    psL = aps.tile([96, KS, 128], F32, tag='L')
                    lhs = q_ext[:, j * 96:(j + 1) * 96]
                    ke = k_ext[:].rearrange('c (r w) -> c r w', w=96)
                    nc.tensor.matmul(psL[:, 0:4, 0:96], lhs,
                                     ke[:, kr0:kr0 + 8:2, :], start=True, stop=True)
                    nc.tensor.matmul(psL[:, 4:7, 0:96], lhs,
                                     ke[:, kr0 + 8:kr0 + 14:2, :], start=True, stop=True)
                    lm = att.tile([96, KS, 96], F32, tag='lm')
                    nc.vector.tensor_tensor(out=lm[:], in0=psL[:, :, 0:96],
                                            in1=wb_b, op=mybir.AluOpType.add)
                    pexp = att.tile([96, KS, 96], BF16, tag='p')
                    den = att.tile([96, 1], F32, tag='den')
                    nc.scalar.activation(out=pexp[:], in_=lm[:],
                                         func=mybir.ActivationFunctionType.Exp,
                                         accum_out=den[:])
                    rden = att.tile([96, 1], F32, tag='rden')
                    nc.vector.reciprocal(out=rden[:], in_=den[:])
                    psT = tp2.tile([96, KS, 128], BF16, tag='T')
                    for i in range(KS):
                        nc.tensor.transpose(psT[:, i, 0:96], pexp[:, i, :],
                                            ident[0:96, 0:96])
                    pt = att.tile([96, KS, 96], BF16, tag='pt')
                    if j % 2 == 0:
                        nc.scalar.copy(pt[:], psT[:, :, 0:96])
                    else:
                        nc.vector.tensor_copy(out=pt[:], in_=psT[:, :, 0:96])
                    psO = ops.tile([96, 128], F32, tag='O')
                    for i in range(KS):
                        r = kr0 + 2 * i
                        nc.tensor.matmul(psO[:], pt[:, i, :],
                                         vt[:, r * 128:(r + 1) * 128],
                                         start=(i == 0), stop=(i == KS - 1))
                    oh = att.tile([96, 128], F32, tag='oh')
                    nc.vector.tensor_scalar_mul(oh[:], psO[:], rden[:])
                    nc.sync.dma_start(out=o[j], in_=oh[:])

    _split_excess_waits(nc)
    _CACHE['nc'] = nc
    return nc


# ---------------------------------------------------------------- kernel ---
def _make_in_maps(x, wq, bq, wk, bk, wv, bv):
    x = np.asarray(x, dtype=np.float32)
    wq = np.asarray(wq, dtype=np.float32)
    wk = np.asarray(wk, dtype=np.float32)
    wv = np.asarray(wv, dtype=np.float32)
    bq = np.asarray(bq, dtype=np.float32)
    bk = np.asarray(bk, dtype=np.float32)
    bv = np.asarray(bv, dtype=np.float32)
    QD, KD = _hdist_channels()
    wbias = _wbias()
    wq_s = wq * SCALE
    w2 = np.zeros((3, 128, 128), dtype=np.float32)
    w1 = np.zeros((3, 64, 128), dtype=np.float32)
    v2 = np.zeros((3, 128, 128), dtype=np.float32)
    v1 = np.zeros((3, 64, 128), dtype=np.float32)
    for kx in range(3):
        w2[kx, 0:64, 0:64] = wq_s[:, :, 0, kx].T
        w2[kx, 0:64, 64:128] = wk[:, :, 0, kx].T
        w2[kx, 64:128, 0:64] = wq_s[:, :, 1, kx].T
        w2[kx, 64:128, 64:128] = wk[:, :, 1, kx].T
        w1[kx, :, 0:64] = wq_s[:, :, 2, kx].T
        w1[kx, :, 64:128] = wk[:, :, 2, kx].T
        v2[kx, 0:64, :] = wv[:, :, 0, kx].T
        v2[kx, 64:128, :] = wv[:, :, 1, kx].T
        v1[kx, :, :] = wv[:, :, 2, kx].T
    qkbias = np.concatenate([bq * SCALE, bk]).reshape(128, 1).astype(np.float32)
    vbias = bv.reshape(128, 1).astype(np.float32)

    in_maps = []
    for core in range(NCORES):
        b, slab = core // NH, core % NH
        h0 = slab * HS
        xsl = np.zeros((64, XR, 98), dtype=np.float32)
        r_lo, r_hi = h0 - 13, h0 + 37  # image rows of slab
        src_lo, src_hi = max(0, r_lo), min(H, r_hi)
        xsl[:, src_lo - r_lo: src_hi - r_lo, 1:97] = x[b, :, src_lo:src_hi, :]
        qdf = np.repeat(QD[:, h0:h0 + HS, None], 96, axis=2).reshape(NDIST, -1)
        kdf = np.zeros((NDIST, KV, 96), dtype=np.float32)
        for r in range(KV):
            img = h0 - 12 + r
            kdf[:, r, :] = KD[:, img % H, None]  # out-of-range rows are masked
        in_maps.append({
            'xs': xsl, 'wqk2': w2, 'wqk1': w1, 'wv2': v2, 'wv1': v1,
            'qkb': qkbias, 'vb': vbias,
            'qd': np.ascontiguousarray(qdf, dtype=np.float32),
            'kd': np.ascontiguousarray(kdf.reshape(NDIST, -1)),
            'wbt': wbias,
        })
    return in_maps


def kernel(x, wq, bq, wk, bk, wv, bv):
    x = np.asarray(x, dtype=np.float32)
    wq = np.asarray(wq, dtype=np.float32)
    wk = np.asarray(wk, dtype=np.float32)
    wv = np.asarray(wv, dtype=np.float32)
    bq = np.asarray(bq, dtype=np.float32)
    bk = np.asarray(bk, dtype=np.float32)
    bv = np.asarray(bv, dtype=np.float32)

    nc = _build_program()
    in_maps = _make_in_maps(x=x, wq=wq, bq=bq, wk=wk, bk=bk, wv=wv, bv=bv)

    from concourse.bass_utils import run_bass_kernel_spmd
    res = run_bass_kernel_spmd(nc, in_maps, core_ids=list(range(NCORES)))
    globals()['_LAST_RES'] = res

    out = np.zeros((B, H, W, CO), dtype=np.float32)
    for core in range(NCORES):
        b, slab = core // NH, core % NH
        out[b, slab * HS:(slab + 1) * HS] = res.results[core]['o']

    border = _host_border(x, wq, bq, wk, bk, wv, bv)
    for h, val in border.items():
        out[:, h] = val.astype(np.float32)
    return out



# revision 9
# speedup vs baseline: 1.4144x; 1.4144x over previous
"""ConvNAT (conv QKV + 2D dilated neighborhood attention) on 8 trn2 cores.

Sharding: core = (batch b, H-slab of 24 rows).  Each core computes conv
q/k/v for the 36 kv rows its attention needs (q rows + 12-row halo) and
the attention for its 24 output rows.  H-border rows (h<6, h>=90), whose
NATTEN windows are clamped and would break SPMD program uniformity, are
computed on the host in numpy and overwrite the device output.

Everything on device is bf16 through the PE (fp32r runs the array at
half clock), psum accumulation in f32.

Attention (r-major, transposed logits):
  for each key row r: LT[kc, jslot, q] = K_ext_r^T Q_ext  (bf16 matmuls)
    K_ext/Q_ext = [conv channels (64) | 7 "h-distance" channels that
    reproduce scale*pe_h.pe_h(h-h') exactly for |h-h'|<=6], q side
    pre-scaled by 128^-0.5.  Keys land on psum PARTITIONS, so P needs
    no transpose before AV.
  ACT exp -> DVE multiply by E[kc,q] = exp(scale*pe_w.pe_w + mask)
    (0 for non-neighbor w columns) -> pm (bf16).
  AV per query row j: psO[q, 129] = sum_i pm_i^T V^T_i, where V^T has a
    ones-column at 128 giving the softmax denominator; DVE divides
    during psum evacuation.
"""
import os
import re
import sys

sys.path.insert(0, '/opt/trn_rl_repo')

import numpy as np
import ml_dtypes

import concourse.bass as bass
from concourse import mybir
from concourse.tile import TileContext
from concourse.masks import make_identity
from bass_rust import ScopedClock, VectorClock

F32 = mybir.dt.float32
BF16 = mybir.dt.bfloat16
BF16NP = ml_dtypes.bfloat16

B, CIN, H, W = 2, 64, 96, 96
CI, CO = 64, 128
KS, DIL = 7, 2
SCALE = float(CI * 2) ** -0.5  # Cqk = 128 after pe concat
HS = 24          # rows per core
NH = 4           # h-slabs
NCORES = 8
NDIST = 7        # h-distance channels
CH = 64 + NDIST  # contraction channels
NKV = 36         # k/v rows per core (24 + 6 halo each side, kv idx 0..35)
XR = 50          # x slab rows
VP = 132         # vt row pitch (128 ch + ones col + pad)
NEG = -30000.0

# ---------------------------------------------------------------- compat ---
MAX_WAITS = 1


def _patched_drain(self, tick_clock, wait_clock):
    nc = self.nc
    ticks = [int(v) for v in re.findall(r'\d+', repr(tick_clock.global_clock))]
    for i in range(0, len(ticks), MAX_WAITS):
        chunk = [0] * len(ticks)
        chunk[i:i + MAX_WAITS] = ticks[i:i + MAX_WAITS]
        if any(chunk):
            probe = nc.sync.nop()
            wait_clock.add_sem_waits(probe.ins, ScopedClock({None: VectorClock(chunk)}))
    nc.sync.drain()
    nc.all_engine_barrier()
    popped = nc._tile_sem_poison_stack.pop()
    assert popped is self._sem_poison
    nc.clear_and_free_semaphores(list(self.sems.allocated().values()))
    nc.all_engine_barrier()


TileContext._drain_and_barrier = _patched_drain


def _split_excess_waits(nc, max_waits=MAX_WAITS):
    n_split = 0
    for fn in nc.m.functions:
        for bb in fn.blocks:
            out = []
            changed = False
            for inst in bb.instructions:
                si = inst.sync_info
                waits = list(si.on_wait) if si and si.on_wait else []
                if len(waits) > max_waits:
                    extra = waits[:-max_waits]
                    for j in range(0, len(extra), max_waits):
                        nop = mybir.InstNoOp(name=f"{inst.name}-ws{j}", ins=[], outs=[])
                        nop.engine = inst.engine
                        nop.sync_info = mybir.SyncInfo(
                            on_wait=extra[j:j + max_waits], on_update=[])
                        out.append(nop)
                    si.on_wait = waits[-max_waits:]
                    changed = True
                    n_split += 1
                out.append(inst)
            if changed:
                bb.instructions = out
    return n_split


# ------------------------------------------------------------- host math ---
def _sincos(length, dim):
    half = dim // 2
    inv_freq = 1.0 / (10000.0 ** (np.arange(half, dtype=np.float64) * 2.0 / dim))
    ang = np.arange(length, dtype=np.float64)[:, None] * inv_freq[None, :]
    return np.concatenate([np.sin(ang), np.cos(ang)], axis=-1)  # (L, dim)


def _na_indices(L, K, D):
    i = np.arange(L)
    g = i % D
    r = i // D
    Lg = (L - g + D - 1) // D
    start = np.clip(r - K // 2, 0, Lg - K)
    return g[:, None] + (start[:, None] + np.arange(K)[None, :]) * D  # (L, K)


def _hdist_channels():
    """QD,KD (NDIST, 96): sum_m QD[m,h]*KD[m,h'] == SCALE*pe_h[h].pe_h[h']
    exactly for even |h-h'| <= 6."""
    pe = _sincos(H, 32)           # (96, 32)
    inv_freq = 1.0 / (10000.0 ** (np.arange(16, dtype=np.float64) * 2.0 / 32))
    dv = np.array([0., 2., 4., 6.])
    g = SCALE * np.cos(dv[:, None] * inv_freq[None, :]).sum(1)  # exact pe.pe(d)
    th = np.arange(4, dtype=np.float64) * (np.pi / 6.0)
    M = np.cos(dv[:, None] * th[None, :])                        # (4, 4)
    b = np.linalg.solve(M, g)
    hh = np.arange(H, dtype=np.float64)
    QD = np.zeros((NDIST, H))
    KD = np.zeros((NDIST, H))
    QD[0] = b[0]
    KD[0] = 1.0
    for m in range(1, 4):
        QD[2 * m - 1] = b[m] * np.cos(th[m] * hh)
        QD[2 * m] = b[m] * np.sin(th[m] * hh)
        KD[2 * m - 1] = np.cos(th[m] * hh)
        KD[2 * m] = np.sin(th[m] * hh)
    # verify
    got = QD.T @ KD
    ref = SCALE * (pe @ pe.T)
    for d in (-6, -4, -2, 0, 2, 4, 6):
        idx = np.arange(max(0, -d), min(H, H - d))
        err = np.abs(got[idx, idx + d] - ref[idx, idx + d]).max()
        assert err < 1e-6, (d, err)
    return QD.astype(np.float32), KD.astype(np.float32)


def _wbias():
    pe = _sincos(W, 32)
    idx_w = _na_indices(W, KS, DIL)   # (96, 7)
    wb = np.full((W, W), NEG, dtype=np.float64)
    dot = SCALE * (pe @ pe.T)
    for w in range(W):
        wb[w, idx_w[w]] = dot[w, idx_w[w]]
    return wb  # (96 q, 96 kc), float64


def _conv_np(x, w, bias, rows):
    """NCHW 3x3 pad-1 conv evaluated at `rows` -> (B, len(rows), 96, Cout)."""
    Bn, Cin, Hn, Wn = x.shape
    xp = np.zeros((Bn, Cin, Hn + 2, Wn + 2), dtype=np.float64)
    xp[:, :, 1:-1, 1:-1] = x
    rows = np.asarray(rows)
    acc = np.zeros((Bn, len(rows), Wn, w.shape[0]), dtype=np.float64)
    for ky in range(3):
        for kx in range(3):
            xs = xp[:, :, rows + ky, :][:, :, :, kx:kx + Wn]  # (B,C,R,W)
            acc += np.einsum('bcrw,oc->brwo', xs, w[:, :, ky, kx].astype(np.float64))
    return acc + bias[None, None, None, :].astype(np.float64)


def _host_border(x, wq, bq, wk, bk, wv, bv):
    """Reference computation for the clamped border rows. -> dict h -> (B,96,128)."""
    border_h = list(range(0, 6)) + list(range(90, 96))
    kv_rows = sorted(set(np.concatenate([_na_indices(H, KS, DIL)[h] for h in border_h])))
    kv_rows = np.asarray(kv_rows)
    q_c = _conv_np(x, wq, bq, np.asarray(border_h))     # (B, 12, 96, 64)
    k_c = _conv_np(x, wk, bk, kv_rows)                  # (B, R, 96, 64)
    v_c = _conv_np(x, wv, bv, kv_rows)                  # (B, R, 96, 128)
    kv_pos = {r: i for i, r in enumerate(kv_rows)}
    pe_h = _sincos(H, 32)
    pe_w = _sincos(W, 32)
    idx_h = _na_indices(H, KS, DIL)
    idx_w = _na_indices(W, KS, DIL)
    out = {}
    for bi, h in enumerate(border_h):
        pe_q = np.concatenate([np.repeat(pe_h[h][None], W, 0), pe_w], axis=1)  # (96,64)
        q = np.concatenate([q_c[:, bi], np.repeat(pe_q[None], B, 0)], axis=2)  # (B,96,128)
        rows = [kv_pos[r] for r in idx_h[h]]
        kk = k_c[:, rows]                                   # (B,7,96,64)
        vv = v_c[:, rows]                                   # (B,7,96,128)
        pe_k = np.concatenate(
            [np.repeat(pe_h[idx_h[h]][:, None, :], W, 1),
             np.repeat(pe_w[None], KS, 0)], axis=2)         # (7,96,64)
        kk = np.concatenate([kk, np.repeat(pe_k[None], B, 0)], axis=3)  # (B,7,96,128)
        kn = kk[:, :, idx_w]                                # (B,7,96,7,128)
        vn = vv[:, :, idx_w]
        logits = SCALE * np.einsum('bwc,biwjc->bwij', q, kn)   # (B,96,7,7)
        m = logits.reshape(B, W, -1).max(-1)
        p = np.exp(logits - m[:, :, None, None])
        p /= p.reshape(B, W, -1).sum(-1)[:, :, None, None]
        out[h] = np.einsum('bwij,biwjc->bwc', p, vn)        # (B,96,128)
    return out


# ------------------------------------------------------------ bass build ---
_CACHE = {}


def _build_program():
    if 'nc' in _CACHE:
        return _CACHE['nc']
    nc = bass.Bass('TRN2')
    xs = nc.dram_tensor('xs', (64, XR, 98), BF16, kind='ExternalInput')
    wqk = nc.dram_tensor('wqk', (5, 128, 128), BF16, kind='ExternalInput')
    wv = nc.dram_tensor('wv', (5, 128, 128), BF16, kind='ExternalInput')
    qkb = nc.dram_tensor('qkb', (128, 1), F32, kind='ExternalInput')
    vb = nc.dram_tensor('vb', (128, 1), F32, kind='ExternalInput')
    qd = nc.dram_tensor('qd', (NDIST, HS * 96), BF16, kind='ExternalInput')
    kd = nc.dram_tensor('kd', (NDIST, NKV * 96), BF16, kind='ExternalInput')
    et = nc.dram_tensor('et', (96, 96), BF16, kind='ExternalInput')
    o = nc.dram_tensor('o', (HS, 96, 128), F32, kind='ExternalOutput')

    with TileContext(nc) as tc:
        with tc.tile_pool(name='persist', bufs=1) as pp:
            # x copies: x2 = [x | x shifted +1 row]; x3 = [x +2 rows | x +2 rows +1 col]
            x2 = pp.tile([128, XR, 98], BF16)
            nc.sync.dma_start(out=x2[0:64], in_=xs[:])
            nc.sync.dma_start(out=x2[64:128, 0:XR - 1, :], in_=xs[:, 1:XR, :])
            x3 = pp.tile([128, XR - 2, 98], BF16)
            nc.sync.dma_start(out=x3[0:64], in_=xs[:, 2:XR, :])
            nc.sync.dma_start(out=x3[64:128, :, 0:97], in_=xs[:, 2:XR, 1:98])
            w_qk = pp.tile([128, 5, 128], BF16)
            nc.sync.dma_start(out=w_qk, in_=wqk[:].rearrange('t p n -> p t n'))
            w_v = pp.tile([128, 5, 128], BF16)
            nc.sync.dma_start(out=w_v, in_=wv[:].rearrange('t p n -> p t n'))
            qkbias = pp.tile([128, 1], F32)
            nc.sync.dma_start(out=qkbias, in_=qkb[:])
            vbias = pp.tile([128, 1], F32)
            nc.sync.dma_start(out=vbias, in_=vb[:])
            ee = pp.tile([96, 96], BF16)
            nc.sync.dma_start(out=ee, in_=et[:])
            ident = pp.tile([128, 128], BF16)
            make_identity(nc, ident)

            q_ext = pp.tile([CH, HS, 96], BF16)
            k_ext = pp.tile([CH, NKV, 128], BF16)  # cols 96:128 junk (never read)
            vsb = pp.tile([128, NKV, 96], BF16)
            vt = pp.tile([96, NKV, VP], BF16)
            nc.sync.dma_start(
                out=q_ext[64:CH], in_=qd[:].rearrange('c (r w) -> c r w', w=96))
            nc.sync.dma_start(
                out=k_ext[64:CH, :, 0:96], in_=kd[:].rearrange('c (r w) -> c r w', w=96))
            # ones column for the softmax denominator
            nc.vector.memset(vt[:, :, 128:129], 1.0)

            # pm[kc, kvplane, slot, q]: P^T blocks; plane = j + 2*i (= kv idx),
            # slot = 6 - i.  Split by plane%4 to keep write/read deps loose.
            pms = [pp.tile([96, 9, 7, 128], BF16, name=f'pm{i}') for i in range(4)]

            # ------------------------------------------------ convolution --
            # kv idx 0..35 = image rows h0-6 .. h0+29; conv for kv idx m uses
            # x-slab rows m+6, m+7, m+8 (ky=0,1,2); x slab row 0 = image h0-13.
            # Passes: p0..p2 = (ky0,ky1) pair at kx=p via x2; p3 = (ky2,kx0)+
            # (ky2,kx1) via x3; p4 = (ky2,kx2), K=64.
            for which in range(2):  # 0 = qk, 1 = v
                wsb = w_qk if which == 0 else w_v
                for chunk in range(3):
                    with tc.tile_pool(name='cps', bufs=6, space='PSUM') as cps:
                        psl = [cps.tile([128, 4, 96], F32, tag='c', name=f'c{g}')
                               for g in range(3)]
                        for p in range(5):
                            for g in range(3):
                                r0 = 6 + chunk * 12 + g * 4  # x-slab row
                                if p < 3:
                                    mov = x2[:, r0:r0 + 4, p:p + 96]
                                    wst = wsb[:, p, :]
                                elif p == 3:
                                    mov = x3[:, r0:r0 + 4, 0:96]
                                    wst = wsb[:, 3, :]
                                else:
                                    mov = x3[0:64, r0:r0 + 4, 2:98]
                                    wst = wsb[0:64, 4, :]
                                nc.tensor.matmul(psl[g][:], wst, mov,
                                                 start=(p == 0), stop=(p == 4))
                        for g in range(3):
                            m0 = chunk * 12 + g * 4  # kv idx of group start
                            if which == 0:
                                # q rows: kv idx 6..29 <-> q row m-6
                                qlo, qhi = max(m0, 6), min(m0 + 4, 30)
                                if qlo < qhi:
                                    nc.scalar.activation(
                                        out=q_ext[0:64, qlo - 6:qhi - 6, :],
                                        in_=psl[g][0:64, qlo - m0:qhi - m0, :],
                                        func=mybir.ActivationFunctionType.Identity,
                                        bias=qkbias[0:64])
                                nc.vector.tensor_scalar(
                                    out=k_ext[0:64, m0:m0 + 4, 0:96],
                                    in0=psl[g][64:128], scalar1=qkbias[64:128],
                                    scalar2=None, op0=mybir.AluOpType.add)
                            else:
                                if g % 2 == 0:
                                    nc.scalar.activation(
                                        out=vsb[:, m0:m0 + 4, :],
                                        in_=psl[g][:],
                                        func=mybir.ActivationFunctionType.Identity,
                                        bias=vbias[:])
                                else:
                                    nc.vector.tensor_scalar(
                                        out=vsb[:, m0:m0 + 4, :],
                                        in0=psl[g][:], scalar1=vbias[:],
                                        scalar2=None, op0=mybir.AluOpType.add)

            # ------------------------------------------------- V^T --------
            with tc.tile_pool(name='tps', bufs=2, space='PSUM') as tps:
                for grp in range(9):
                    pst = tps.tile([96, 4, 128], BF16, tag='t')
                    for rr in range(4):
                        m = grp * 4 + rr
                        nc.tensor.transpose(pst[:, rr, :], vsb[:, m, :], ident)
                    if grp % 2 == 0:
                        nc.scalar.copy(vt[:, grp * 4:(grp + 1) * 4, 0:128], pst[:])
                    else:
                        nc.vector.tensor_copy(
                            out=vt[:, grp * 4:(grp + 1) * 4, 0:128], in_=pst[:])

            # ---------------------------------------------- attention -----
            # E broadcast over the slot dim
            ee_ap = ee[:, :]

            def ee_b(n):
                return bass.AP(tensor=ee_ap.tensor, offset=ee_ap.offset,
                               ap=[ee_ap.ap[0], [0, n], ee_ap.ap[1]])

            qv = q_ext[:, :, :]  # [CH, HS, 96]

            def do_av(j):
                psO = ops.tile([128, 129], F32, tag='O')
                for i in range(KS):
                    plane = j + 2 * i
                    pm = pms[plane % 4]
                    nc.tensor.matmul(psO[:], pm[:, plane // 4, 6 - i, :],
                                     vt[:, plane, 0:129],
                                     start=(i == 0), stop=(i == KS - 1))
                rden = att.tile([96, 1], F32, tag='rden')
                nc.vector.reciprocal(out=rden[:], in_=psO[0:96, 128:129])
                oh = att.tile([96, 128], F32, tag='oh')
                nc.vector.tensor_scalar_mul(oh[:], psO[0:96, 0:128], rden[:])
                nc.sync.dma_start(out=o[j], in_=oh[:])

            with tc.tile_pool(name='aps', bufs=2, space='PSUM') as aps, \
                 tc.tile_pool(name='ops', bufs=2, space='PSUM') as ops, \
                 tc.tile_pool(name='att', bufs=3) as att:
                for m in range(NKV):  # kv idx; key image row = h0 - 6 + m
                    jav = m - 16  # AV for query row ready 4 iters ago
                    if 0 <= jav < HS:
                        do_av(jav)
                    # query rows attending key row m: j = m - 12 + 2t, t=0..6
                    t0 = max(0, (13 - m) // 2)          # j >= 0
                    t1 = min(6, (HS - 1 - m + 12) // 2)  # j <= 23
                    nt = t1 - t0 + 1
                    nA = min(nt, 4)
                    nB = nt - nA
                    jA = m - 12 + 2 * t0
                    kst = k_ext[:, m, :]  # [CH, 128] stationary
                    psA = aps.tile([128, 4, 96], F32, tag='A')
                    qa = qv[:, jA:jA + 2 * nA - 1:2, :]
                    nc.tensor.matmul(psA[:, 0:nA, :], kst, qa, start=True, stop=True)
                    psB = aps.tile([128, 3, 96], F32, tag='B')
                    if nB > 0:
                        jB = jA + 2 * nA
                        qb = qv[:, jB:jB + 2 * nB - 1:2, :]
                        nc.tensor.matmul(psB[:, 0:nB, :], kst, qb,
                                         start=True, stop=True)
                    tmp = att.tile([96, 7, 96], BF16, tag='tmp')
                    nc.scalar.activation(out=tmp[:, 0:nA, :], in_=psA[0:96, 0:nA, :],
                                         func=mybir.ActivationFunctionType.Exp)
                    if nB > 0:
                        nc.scalar.activation(out=tmp[:, nA:nt, :],
                                             in_=psB[0:96, 0:nB, :],
                                             func=mybir.ActivationFunctionType.Exp)
                    pm = pms[m % 4]
                    nc.vector.tensor_tensor(
                        out=pm[:, m // 4, t0:t1 + 1, 0:96], in0=tmp[:, 0:nt, :],
                        in1=ee_b(nt), op=mybir.AluOpType.mult)
                for jav in range(NKV - 16, HS):
                    do_av(jav)

    _split_excess_waits(nc)
    _CACHE['nc'] = nc
    return nc


# ---------------------------------------------------------------- kernel ---
def _make_in_maps(x, wq, bq, wk, bk, wv, bv):
    x = np.asarray(x, dtype=np.float32)
    wq = np.asarray(wq, dtype=np.float32)
    wk = np.asarray(wk, dtype=np.float32)
    wv = np.asarray(wv, dtype=np.float32)
    bq = np.asarray(bq, dtype=np.float32)
    bk = np.asarray(bk, dtype=np.float32)
    bv = np.asarray(bv, dtype=np.float32)
    QD, KD = _hdist_channels()
    wbias = _wbias()
    ebias = np.exp(wbias).T.astype(BF16NP)  # E[kc, q] = exp(wb[q, kc])
    wq_s = wq * SCALE

    def pack5(w, scale_first64=None):
        # returns (5, 128, 128): p0..p2 = (ky0|ky1) at kx=p; p3 = ky2 kx0|kx1;
        # p4 rows 0:64 = ky2 kx2
        out = np.zeros((5, 128, 128), dtype=np.float32)
        for kx in range(3):
            out[kx, 0:64, :] = w[:, :, 0, kx].T
            out[kx, 64:128, :] = w[:, :, 1, kx].T
        out[3, 0:64, :] = w[:, :, 2, 0].T
        out[3, 64:128, :] = w[:, :, 2, 1].T
        out[4, 0:64, :] = w[:, :, 2, 2].T
        return out

    # build a (cout=128) combined weight: cols 0:64 = q(scaled), 64:128 = k
    wcomb = np.zeros((128, 64, 3, 3), dtype=np.float32)
    wcomb[0:64] = wq_s
    wcomb[64:128] = wk
    # pack expects w[cout, cin, ky, kx] with .T -> [cin, cout]
    wqk_p = pack5(wcomb)
    wv_p = pack5(wv)
    qkbias = np.concatenate([bq * SCALE, bk]).reshape(128, 1).astype(np.float32)
    vbias = bv.reshape(128, 1).astype(np.float32)

    in_maps = []
    for core in range(NCORES):
        b, slab = core // NH, core % NH
        h0 = slab * HS
        xsl = np.zeros((64, XR, 98), dtype=np.float32)
        r_lo, r_hi = h0 - 13, h0 + 37  # image rows of slab
        src_lo, src_hi = max(0, r_lo), min(H, r_hi)
        xsl[:, src_lo - r_lo: src_hi - r_lo, 1:97] = x[b, :, src_lo:src_hi, :]
        qdf = np.repeat(QD[:, h0:h0 + HS, None], 96, axis=2).reshape(NDIST, -1)
        kdf = np.zeros((NDIST, NKV, 96), dtype=np.float32)
        for m in range(NKV):
            img = h0 - 6 + m
            kdf[:, m, :] = KD[:, img % H, None]  # out-of-range rows are masked
        in_maps.append({
            'xs': xsl.astype(BF16NP),
            'wqk': wqk_p.astype(BF16NP), 'wv': wv_p.astype(BF16NP),
            'qkb': qkbias, 'vb': vbias,
            'qd': np.ascontiguousarray(qdf).astype(BF16NP),
            'kd': np.ascontiguousarray(kdf.reshape(NDIST, -1)).astype(BF16NP),
            'et': ebias,
        })
    return in_maps


def kernel(x, wq, bq, wk, bk, wv, bv):
    x = np.asarray(x, dtype=np.float32)
    wq = np.asarray(wq, dtype=np.float32)
    wk = np.asarray(wk, dtype=np.float32)
    wv = np.asarray(wv, dtype=np.float32)
    bq = np.asarray(bq, dtype=np.float32)
    bk = np.asarray(bk, dtype=np.float32)
    bv = np.asarray(bv, dtype=np.float32)

    nc = _build_program()
    in_maps = _make_in_maps(x=x, wq=wq, bq=bq, wk=wk, bk=bk, wv=wv, bv=bv)

    from concourse.bass_utils import run_bass_kernel_spmd
    res = run_bass_kernel_spmd(nc, in_maps, core_ids=list(range(NCORES)))
    globals()['_LAST_RES'] = res

    out = np.zeros((B, H, W, CO), dtype=np.float32)
    for core in range(NCORES):
        b, slab = core // NH, core % NH
        out[b, slab * HS:(slab + 1) * HS] = res.results[core]['o']

    border = _host_border(x, wq, bq, wk, bk, wv, bv)
    for h, val in border.items():
        out[:, h] = val.astype(np.float32)
    return out


# revision 14
# speedup vs baseline: 1.8802x; 1.3293x over previous
"""ConvNAT (conv QKV + 2D dilated neighborhood attention) on 8 trn2 cores.

Sharding: core = (batch b, H-slab of 24 rows).  Each core computes conv
q/k/v for the 36 kv rows its attention needs (q rows + 12-row halo) and
the attention for its 24 output rows.  H-border rows (h<6, h>=90), whose
NATTEN windows are clamped and would break SPMD program uniformity, are
computed on the host in numpy and overwrite the device output.

Everything on device is bf16 through the PE (fp32r runs the array at
half clock), psum accumulation in f32.

Attention (r-major, transposed logits):
  for each key row r: LT[kc, jslot, q] = K_ext_r^T Q_ext  (bf16 matmuls)
    K_ext/Q_ext = [conv channels (64) | 7 "h-distance" channels that
    reproduce scale*pe_h.pe_h(h-h') exactly for |h-h'|<=6], q side
    pre-scaled by 128^-0.5.  Keys land on psum PARTITIONS, so P needs
    no transpose before AV.
  ACT exp -> DVE multiply by E[kc,q] = exp(scale*pe_w.pe_w + mask)
    (0 for non-neighbor w columns) -> pm (bf16).
  AV per query row j: psO[q, 129] = sum_i pm_i^T V^T_i, where V^T has a
    ones-column at 128 giving the softmax denominator; DVE divides
    during psum evacuation.
"""
import os
import re
import sys

sys.path.insert(0, '/opt/trn_rl_repo')

import numpy as np
import ml_dtypes

import concourse.bass as bass
from concourse import mybir
from concourse.tile import TileContext
from concourse.masks import make_identity
from bass_rust import ScopedClock, VectorClock

F32 = mybir.dt.float32
BF16 = mybir.dt.bfloat16
BF16NP = ml_dtypes.bfloat16

B, CIN, H, W = 2, 64, 96, 96
CI, CO = 64, 128
KS, DIL = 7, 2
SCALE = float(CI * 2) ** -0.5  # Cqk = 128 after pe concat
HS = 24          # rows per core
NH = 4           # h-slabs
NCORES = 8
NDIST = 7        # h-distance channels
CH = 64 + NDIST  # contraction channels
NKV = 36         # k/v rows per core (24 + 6 halo each side, kv idx 0..35)
XR = 50          # x slab rows
VP = 132         # vt row pitch (128 ch + ones col + pad)
NEG = -30000.0

# ---------------------------------------------------------------- compat ---
MAX_WAITS = 1


def _patched_drain(self, tick_clock, wait_clock):
    nc = self.nc
    ticks = [int(v) for v in re.findall(r'\d+', repr(tick_clock.global_clock))]
    for i in range(0, len(ticks), MAX_WAITS):
        chunk = [0] * len(ticks)
        chunk[i:i + MAX_WAITS] = ticks[i:i + MAX_WAITS]
        if any(chunk):
            probe = nc.sync.nop()
            wait_clock.add_sem_waits(probe.ins, ScopedClock({None: VectorClock(chunk)}))
    nc.sync.drain()
    nc.all_engine_barrier()
    popped = nc._tile_sem_poison_stack.pop()
    assert popped is self._sem_poison
    nc.clear_and_free_semaphores(list(self.sems.allocated().values()))
    nc.all_engine_barrier()


TileContext._drain_and_barrier = _patched_drain


def _split_excess_waits(nc, max_waits=MAX_WAITS):
    n_split = 0
    for fn in nc.m.functions:
        for bb in fn.blocks:
            out = []
            changed = False
            for inst in bb.instructions:
                si = inst.sync_info
                waits = list(si.on_wait) if si and si.on_wait else []
                if len(waits) > max_waits:
                    extra = waits[:-max_waits]
                    for j in range(0, len(extra), max_waits):
                        nop = mybir.InstNoOp(name=f"{inst.name}-ws{j}", ins=[], outs=[])
                        nop.engine = inst.engine
                        nop.sync_info = mybir.SyncInfo(
                            on_wait=extra[j:j + max_waits], on_update=[])
                        out.append(nop)
                    si.on_wait = waits[-max_waits:]
                    changed = True
                    n_split += 1
                out.append(inst)
            if changed:
                bb.instructions = out
    return n_split


# ------------------------------------------------------------- host math ---
def _sincos(length, dim):
    half = dim // 2
    inv_freq = 1.0 / (10000.0 ** (np.arange(half, dtype=np.float64) * 2.0 / dim))
    ang = np.arange(length, dtype=np.float64)[:, None] * inv_freq[None, :]
    return np.concatenate([np.sin(ang), np.cos(ang)], axis=-1)  # (L, dim)


def _na_indices(L, K, D):
    i = np.arange(L)
    g = i % D
    r = i // D
    Lg = (L - g + D - 1) // D
    start = np.clip(r - K // 2, 0, Lg - K)
    return g[:, None] + (start[:, None] + np.arange(K)[None, :]) * D  # (L, K)


def _hdist_channels():
    """QD,KD (NDIST, 96): sum_m QD[m,h]*KD[m,h'] == SCALE*pe_h[h].pe_h[h']
    exactly for even |h-h'| <= 6."""
    pe = _sincos(H, 32)           # (96, 32)
    inv_freq = 1.0 / (10000.0 ** (np.arange(16, dtype=np.float64) * 2.0 / 32))
    dv = np.array([0., 2., 4., 6.])
    g = SCALE * np.cos(dv[:, None] * inv_freq[None, :]).sum(1)  # exact pe.pe(d)
    th = np.arange(4, dtype=np.float64) * (np.pi / 6.0)
    M = np.cos(dv[:, None] * th[None, :])                        # (4, 4)
    b = np.linalg.solve(M, g)
    hh = np.arange(H, dtype=np.float64)
    QD = np.zeros((NDIST, H))
    KD = np.zeros((NDIST, H))
    QD[0] = b[0]
    KD[0] = 1.0
    for m in range(1, 4):
        QD[2 * m - 1] = b[m] * np.cos(th[m] * hh)
        QD[2 * m] = b[m] * np.sin(th[m] * hh)
        KD[2 * m - 1] = np.cos(th[m] * hh)
        KD[2 * m] = np.sin(th[m] * hh)
    # verify
    got = QD.T @ KD
    ref = SCALE * (pe @ pe.T)
    for d in (-6, -4, -2, 0, 2, 4, 6):
        idx = np.arange(max(0, -d), min(H, H - d))
        err = np.abs(got[idx, idx + d] - ref[idx, idx + d]).max()
        assert err < 1e-6, (d, err)
    return QD.astype(np.float32), KD.astype(np.float32)


def _wbias():
    pe = _sincos(W, 32)
    idx_w = _na_indices(W, KS, DIL)   # (96, 7)
    wb = np.full((W, W), NEG, dtype=np.float64)
    dot = SCALE * (pe @ pe.T)
    for w in range(W):
        wb[w, idx_w[w]] = dot[w, idx_w[w]]
    return wb  # (96 q, 96 kc), float64


def _conv_np(x, w, bias, rows):
    """NCHW 3x3 pad-1 conv evaluated at `rows` -> (B, len(rows), 96, Cout)."""
    Bn, Cin, Hn, Wn = x.shape
    xp = np.zeros((Bn, Cin, Hn + 2, Wn + 2), dtype=np.float64)
    xp[:, :, 1:-1, 1:-1] = x
    rows = np.asarray(rows)
    acc = np.zeros((Bn, len(rows), Wn, w.shape[0]), dtype=np.float64)
    for ky in range(3):
        for kx in range(3):
            xs = xp[:, :, rows + ky, :][:, :, :, kx:kx + Wn]  # (B,C,R,W)
            acc += np.einsum('bcrw,oc->brwo', xs, w[:, :, ky, kx].astype(np.float64))
    return acc + bias[None, None, None, :].astype(np.float64)


def _host_border(x, wq, bq, wk, bk, wv, bv):
    """Reference computation for the clamped border rows. -> dict h -> (B,96,128)."""
    border_h = list(range(0, 6)) + list(range(90, 96))
    kv_rows = sorted(set(np.concatenate([_na_indices(H, KS, DIL)[h] for h in border_h])))
    kv_rows = np.asarray(kv_rows)
    q_c = _conv_np(x, wq, bq, np.asarray(border_h))     # (B, 12, 96, 64)
    k_c = _conv_np(x, wk, bk, kv_rows)                  # (B, R, 96, 64)
    v_c = _conv_np(x, wv, bv, kv_rows)                  # (B, R, 96, 128)
    kv_pos = {r: i for i, r in enumerate(kv_rows)}
    pe_h = _sincos(H, 32)
    pe_w = _sincos(W, 32)
    idx_h = _na_indices(H, KS, DIL)
    idx_w = _na_indices(W, KS, DIL)
    out = {}
    for bi, h in enumerate(border_h):
        pe_q = np.concatenate([np.repeat(pe_h[h][None], W, 0), pe_w], axis=1)  # (96,64)
        q = np.concatenate([q_c[:, bi], np.repeat(pe_q[None], B, 0)], axis=2)  # (B,96,128)
        rows = [kv_pos[r] for r in idx_h[h]]
        kk = k_c[:, rows]                                   # (B,7,96,64)
        vv = v_c[:, rows]                                   # (B,7,96,128)
        pe_k = np.concatenate(
            [np.repeat(pe_h[idx_h[h]][:, None, :], W, 1),
             np.repeat(pe_w[None], KS, 0)], axis=2)         # (7,96,64)
        kk = np.concatenate([kk, np.repeat(pe_k[None], B, 0)], axis=3)  # (B,7,96,128)
        kn = kk[:, :, idx_w]                                # (B,7,96,7,128)
        vn = vv[:, :, idx_w]
        logits = SCALE * np.einsum('bwc,biwjc->bwij', q, kn)   # (B,96,7,7)
        m = logits.reshape(B, W, -1).max(-1)
        p = np.exp(logits - m[:, :, None, None])
        p /= p.reshape(B, W, -1).sum(-1)[:, :, None, None]
        out[h] = np.einsum('bwij,biwjc->bwc', p, vn)        # (B,96,128)
    return out


# ------------------------------------------------------------ bass build ---
_CACHE = {}


def _build_program():
    if 'nc' in _CACHE:
        return _CACHE['nc']
    nc = bass.Bass('TRN2')
    xs = nc.dram_tensor('xs', (64, XR, 98), BF16, kind='ExternalInput')
    wqk = nc.dram_tensor('wqk', (5, 128, 128), BF16, kind='ExternalInput')
    wv = nc.dram_tensor('wv', (5, 128, 128), BF16, kind='ExternalInput')
    qkb = nc.dram_tensor('qkb', (128, 1), F32, kind='ExternalInput')
    vb = nc.dram_tensor('vb', (128, 1), F32, kind='ExternalInput')
    qd = nc.dram_tensor('qd', (NDIST, HS * 96), BF16, kind='ExternalInput')
    kd = nc.dram_tensor('kd', (NDIST, NKV * 96), BF16, kind='ExternalInput')
    et = nc.dram_tensor('et', (96, 96), BF16, kind='ExternalInput')
    o = nc.dram_tensor('o', (HS, 96, 128), F32, kind='ExternalOutput')

    with TileContext(nc) as tc:
        with tc.tile_pool(name='persist', bufs=1) as pp:
            # x copies: x2 = [x | x shifted +1 row]; x3 = [x +2 rows | x +2 rows
            # +1 col].  Row/col shifts are pure flat offsets: load xs from HBM
            # once, then build the shifted halves with SBUF->SBUF DMAs.
            NXE = XR * 98
            x2 = pp.tile([128, XR, 98], BF16)
            x2f = x2[:, :, :].rearrange('p r c -> p (r c)')
            nc.sync.dma_start(out=x2f[0:64, :], in_=xs[:].rearrange('p r c -> p (r c)'))
            w_qk = pp.tile([128, 5, 128], BF16)
            nc.sync.dma_start(out=w_qk, in_=wqk[:].rearrange('t p n -> p t n'))
            w_v = pp.tile([128, 5, 128], BF16)
            nc.sync.dma_start(out=w_v, in_=wv[:].rearrange('t p n -> p t n'))
            x3 = pp.tile([128, XR - 2, 98], BF16)
            x3f = x3[:, :, :].rearrange('p r c -> p (r c)')
            nc.sync.dma_start(out=x2f[64:128, 0:NXE - 98], in_=x2f[0:64, 98:NXE])
            nc.sync.dma_start(out=x3f[0:64, :], in_=x2f[0:64, 196:NXE])
            nc.sync.dma_start(out=x3f[64:128, 0:NXE - 197], in_=x2f[0:64, 197:NXE])
            ee = pp.tile([96, 96], BF16)
            nc.sync.dma_start(out=ee, in_=et[:])
            qkbias = pp.tile([128, 1], F32)
            nc.sync.dma_start(out=qkbias, in_=qkb[:])
            vbias = pp.tile([128, 1], F32)
            nc.sync.dma_start(out=vbias, in_=vb[:])
            ident = pp.tile([128, 128], BF16)
            make_identity(nc, ident)

            q_ext = pp.tile([CH, HS, 96], BF16)
            k_ext = pp.tile([CH, NKV, 128], BF16)  # cols 96:128 junk (never read)
            vsb = pp.tile([128, NKV, 96], BF16)
            vt = pp.tile([96, NKV, VP], BF16)
            nc.sync.dma_start(
                out=q_ext[64:CH], in_=qd[:].rearrange('c (r w) -> c r w', w=96))
            nc.sync.dma_start(
                out=k_ext[64:CH, :, 0:96], in_=kd[:].rearrange('c (r w) -> c r w', w=96))
            # ones column for the softmax denominator
            nc.vector.memset(vt[:, :, 128:129], 1.0)

            # pm[kc, kvplane, slot, q]: P^T blocks; plane = j + 2*i (= kv idx),
            # slot = 6 - i.  Split by plane%4 to keep write/read deps loose.
            pms = [pp.tile([96, 9, 7, 128], BF16, name=f'pm{i}') for i in range(4)]

            # ------------------------------------------------ convolution --
            # kv idx 0..35 = image rows h0-6 .. h0+29; conv for kv idx m uses
            # x-slab rows m+6, m+7, m+8 (ky=0,1,2); x slab row 0 = image h0-13.
            # Passes: p0..p2 = (ky0,ky1) pair at kx=p via x2; p3 = (ky2,kx0)+
            # (ky2,kx1) via x3; p4 = (ky2,kx2), K=64.
            with tc.tile_pool(name='cps', bufs=6, space='PSUM') as cps:
                for which in range(2):  # 0 = qk, 1 = v
                    wsb = w_qk if which == 0 else w_v
                    for chunk in range(3):
                        psl = [cps.tile([128, 4, 96], F32, tag='c', name=f'c{g}')
                               for g in range(3)]
                        for p in range(5):
                            for g in range(3):
                                r0 = 6 + chunk * 12 + g * 4  # x-slab row
                                if p < 3:
                                    mov = x2[:, r0:r0 + 4, p:p + 96]
                                    wst = wsb[:, p, :]
                                elif p == 3:
                                    mov = x3[:, r0:r0 + 4, 0:96]
                                    wst = wsb[:, 3, :]
                                else:
                                    mov = x3[0:64, r0:r0 + 4, 2:98]
                                    wst = wsb[0:64, 4, :]
                                nc.tensor.matmul(psl[g][:], wst, mov,
                                                 start=(p == 0), stop=(p == 4))
                        for g in range(3):
                            m0 = chunk * 12 + g * 4  # kv idx of group start
                            if which == 0:
                                # q rows: kv idx 6..29 <-> q row m-6
                                qlo, qhi = max(m0, 6), min(m0 + 4, 30)
                                if qlo < qhi:
                                    nc.scalar.activation(
                                        out=q_ext[0:64, qlo - 6:qhi - 6, :],
                                        in_=psl[g][0:64, qlo - m0:qhi - m0, :],
                                        func=mybir.ActivationFunctionType.Identity,
                                        bias=qkbias[0:64])
                                nc.vector.tensor_scalar(
                                    out=k_ext[0:64, m0:m0 + 4, 0:96],
                                    in0=psl[g][64:128], scalar1=qkbias[64:128],
                                    scalar2=None, op0=mybir.AluOpType.add)
                            else:
                                if g % 2 == 0:
                                    nc.scalar.activation(
                                        out=vsb[:, m0:m0 + 4, :],
                                        in_=psl[g][:],
                                        func=mybir.ActivationFunctionType.Identity,
                                        bias=vbias[:])
                                else:
                                    nc.vector.tensor_scalar(
                                        out=vsb[:, m0:m0 + 4, :],
                                        in0=psl[g][:], scalar1=vbias[:],
                                        scalar2=None, op0=mybir.AluOpType.add)

            # ---------------------------------------------- attention -----
            # E broadcast over the slot dim
            ee_ap = ee[:, :]

            def ee_b(n):
                return bass.AP(tensor=ee_ap.tensor, offset=ee_ap.offset,
                               ap=[ee_ap.ap[0], [0, n], ee_ap.ap[1]])

            qv = q_ext[:, :, :]  # [CH, HS, 96]

            def do_av(j):
                psO = ops.tile([128, 129], F32, tag='O')
                for i in range(KS):
                    plane = j + 2 * i
                    pm = pms[plane % 4]
                    nc.tensor.matmul(psO[:], pm[:, plane // 4, 6 - i, :],
                                     vt[:, plane, 0:129],
                                     start=(i == 0), stop=(i == KS - 1))
                rden = att.tile([96, 1], F32, tag='rden')
                nc.vector.reciprocal(out=rden[:], in_=psO[0:96, 128:129])
                oh = att.tile([96, 128], F32, tag='oh')
                nc.vector.tensor_scalar_mul(oh[:], psO[0:96, 0:128], rden[:])
                nc.sync.dma_start(out=o[j], in_=oh[:])

            with tc.tile_pool(name='tps', bufs=2, space='PSUM') as tps, \
                 tc.tile_pool(name='aps', bufs=2, space='PSUM') as aps, \
                 tc.tile_pool(name='ops', bufs=2, space='PSUM') as ops, \
                 tc.tile_pool(name='att', bufs=3) as att:
                # V^T (psum slot-pitch 128; slots 0:4 and 4:8 in separate banks)
                for grp in range(9):
                    pst = tps.tile([96, 4, 128], BF16, tag='t')
                    for rr in range(4):
                        m = grp * 4 + rr
                        nc.tensor.transpose(pst[:, rr, :], vsb[:, m, :], ident)
                    if grp % 2 == 0:
                        nc.scalar.copy(vt[:, grp * 4:(grp + 1) * 4, 0:128], pst[:])
                    else:
                        nc.vector.tensor_copy(
                            out=vt[:, grp * 4:(grp + 1) * 4, 0:128], in_=pst[:])

                for m in range(NKV):  # kv idx; key image row = h0 - 6 + m
                    jav = m - 16  # AV for query row ready 4 iters ago
                    if 0 <= jav < HS:
                        do_av(jav)
                    # query rows attending key row m: j = m - 12 + 2t, t=0..6
                    t0 = max(0, (13 - m) // 2)          # j >= 0
                    t1 = min(6, (HS - 1 - m + 12) // 2)  # j <= 23
                    nt = t1 - t0 + 1
                    kst = k_ext[:, m, :]  # [CH, 128] stationary
                    # psL slot pitch 128 -> slots 0:4 / 4:7 in separate banks
                    psL = aps.tile([128, 7, 128], F32, tag='L')
                    for (ta, tb) in ((t0, min(t1, 3)), (max(t0, 4), t1)):
                        if ta > tb:
                            continue
                        nn = tb - ta + 1
                        ja = m - 12 + 2 * ta
                        qa = qv[:, ja:ja + 2 * nn - 1:2, :]
                        nc.tensor.matmul(psL[:, ta:tb + 1, 0:96], kst, qa,
                                         start=True, stop=True)
                    tmp = att.tile([96, 7, 96], BF16, tag='tmp')
                    nc.scalar.activation(out=tmp[:, t0:t1 + 1, :],
                                         in_=psL[0:96, t0:t1 + 1, 0:96],
                                         func=mybir.ActivationFunctionType.Exp)
                    pm = pms[m % 4]
                    nc.vector.tensor_tensor(
                        out=pm[:, m // 4, t0:t1 + 1, 0:96], in0=tmp[:, t0:t1 + 1, :],
                        in1=ee_b(t1 - t0 + 1), op=mybir.AluOpType.mult)
                for jav in range(NKV - 16, HS):
                    do_av(jav)

    _split_excess_waits(nc)
    _CACHE['nc'] = nc
    return nc


# ---------------------------------------------------------------- kernel ---
def _make_in_maps(x, wq, bq, wk, bk, wv, bv):
    x = np.asarray(x, dtype=np.float32)
    wq = np.asarray(wq, dtype=np.float32)
    wk = np.asarray(wk, dtype=np.float32)
    wv = np.asarray(wv, dtype=np.float32)
    bq = np.asarray(bq, dtype=np.float32)
    bk = np.asarray(bk, dtype=np.float32)
    bv = np.asarray(bv, dtype=np.float32)
    QD, KD = _hdist_channels()
    wbias = _wbias()
    ebias = np.exp(wbias).T.astype(BF16NP)  # E[kc, q] = exp(wb[q, kc])
    wq_s = wq * SCALE

    def pack5(w, scale_first64=None):
        # returns (5, 128, 128): p0..p2 = (ky0|ky1) at kx=p; p3 = ky2 kx0|kx1;
        # p4 rows 0:64 = ky2 kx2
        out = np.zeros((5, 128, 128), dtype=np.float32)
        for kx in range(3):
            out[kx, 0:64, :] = w[:, :, 0, kx].T
            out[kx, 64:128, :] = w[:, :, 1, kx].T
        out[3, 0:64, :] = w[:, :, 2, 0].T
        out[3, 64:128, :] = w[:, :, 2, 1].T
        out[4, 0:64, :] = w[:, :, 2, 2].T
        return out

    # build a (cout=128) combined weight: cols 0:64 = q(scaled), 64:128 = k
    wcomb = np.zeros((128, 64, 3, 3), dtype=np.float32)
    wcomb[0:64] = wq_s
    wcomb[64:128] = wk
    # pack expects w[cout, cin, ky, kx] with .T -> [cin, cout]
    wqk_p = pack5(wcomb)
    wv_p = pack5(wv)
    qkbias = np.concatenate([bq * SCALE, bk]).reshape(128, 1).astype(np.float32)
    vbias = bv.reshape(128, 1).astype(np.float32)

    in_maps = []
    for core in range(NCORES):
        b, slab = core // NH, core % NH
        h0 = slab * HS
        xsl = np.zeros((64, XR, 98), dtype=np.float32)
        r_lo, r_hi = h0 - 13, h0 + 37  # image rows of slab
        src_lo, src_hi = max(0, r_lo), min(H, r_hi)
        xsl[:, src_lo - r_lo: src_hi - r_lo, 1:97] = x[b, :, src_lo:src_hi, :]
        qdf = np.repeat(QD[:, h0:h0 + HS, None], 96, axis=2).reshape(NDIST, -1)
        kdf = np.zeros((NDIST, NKV, 96), dtype=np.float32)
        for m in range(NKV):
            img = h0 - 6 + m
            kdf[:, m, :] = KD[:, img % H, None]  # out-of-range rows are masked
        in_maps.append({
            'xs': xsl.astype(BF16NP),
            'wqk': wqk_p.astype(BF16NP), 'wv': wv_p.astype(BF16NP),
            'qkb': qkbias, 'vb': vbias,
            'qd': np.ascontiguousarray(qdf).astype(BF16NP),
            'kd': np.ascontiguousarray(kdf.reshape(NDIST, -1)).astype(BF16NP),
            'et': ebias,
        })
    return in_maps


def kernel(x, wq, bq, wk, bk, wv, bv):
    x = np.asarray(x, dtype=np.float32)
    wq = np.asarray(wq, dtype=np.float32)
    wk = np.asarray(wk, dtype=np.float32)
    wv = np.asarray(wv, dtype=np.float32)
    bq = np.asarray(bq, dtype=np.float32)
    bk = np.asarray(bk, dtype=np.float32)
    bv = np.asarray(bv, dtype=np.float32)

    nc = _build_program()
    in_maps = _make_in_maps(x=x, wq=wq, bq=bq, wk=wk, bk=bk, wv=wv, bv=bv)

    from concourse.bass_utils import run_bass_kernel_spmd
    res = run_bass_kernel_spmd(nc, in_maps, core_ids=list(range(NCORES)))
    globals()['_LAST_RES'] = res

    out = np.zeros((B, H, W, CO), dtype=np.float32)
    for core in range(NCORES):
        b, slab = core // NH, core % NH
        out[b, slab * HS:(slab + 1) * HS] = res.results[core]['o']

    border = _host_border(x, wq, bq, wk, bk, wv, bv)
    for h, val in border.items():
        out[:, h] = val.astype(np.float32)
    return out
